# revision 20
# baseline (speedup 1.0000x reference)
"""Trainium2 Bass kernel for the soft-decision-tree ensemble classifier.

Math (per batch row b, tree t):
  zb[t,n]      = x[b] . W[t,n] + bias[t,n]
  log s        = zb - softplus(zb);  log(1-s) = -softplus(zb)
  log_leaf[l]  = sum_{k in path(l)} dir_k * zb_k  -  sum_{k in path(l)} softplus(zb_k)
  leaf_prob    = exp(log_leaf)
  out[b,c]     = sum_t 2*softmax(tw)_t * sum_l leaf_prob[t,l] * softmax(leaf_logits[t,l])_c

Mapping: data-parallel over the batch (B=4096 -> 512 rows per NeuronCore),
node-partition layout ([tree-node, batch] tiles, two 64-node trees per
128-partition tile).  Engine budget per core:

  PE    : stage-1 z=W.x (128 fp16 matmuls), stage-2 path sums (64 fp16
          matmuls against constant +-1 block-diagonal matrices), stage-3
          output accumulation (32 fp16 matmuls).
  ACT   : softplus via Exp then Ln(1+x) batched over 8-tile [128,4096]
          instructions (single pinned Exp+Ln table), leaf-prob Exp per
          [128,1024] PSUM pair, one Exp for the leaf distributions.
  Vector: z+bias copy out of PSUM per pair (bias added via a broadcast
          tensor_tensor, so no separate bias pass), leaf-dist softmax
          normalization, final out copy.

PSUM: pz pair (2 banks, single buffered - freed as soon as the Vector
copy drains it), pp pairs (2x2 banks), out accumulator (1 bank), early
scratch (1 bank) = 8 banks.
"""

import numpy as np

TREE_DEPTH = 6
T, N, D, C = 64, 63, 512, 100
L = 2**TREE_DEPTH          # 64
NPAD = 64                  # nodes padded per tree
TNP = T * NPAD             # 4096
NTILES = TNP // 128        # 32 (two trees per 128-partition tile)
B = 4096
NCORES = 8
BS = B // NCORES           # 512

# column layout of the packed constants tensor [128, 512]
_COL_BIAS = 0      # [128, 32]
_COL_A2 = 288      # [64, 128]
_COL_E2 = 416      # [64, 32]
_COL_TW = 448      # [1, 64]
_CONST_COLS = 512


def _leaf_paths(depth):
    Ll = 2**depth
    idx = np.zeros((Ll, depth), np.int32)
    dr = np.zeros((Ll, depth), np.int32)
    for l in range(Ll):
        node = 0
        for k in range(depth):
            bit = (l >> (depth - 1 - k)) & 1
            idx[l, k] = node
            dr[l, k] = bit
            node = 2 * node + 1 + bit
    return idx, dr


def _path_mats():
    idx, dr = _leaf_paths(TREE_DEPTH)
    mdir = np.zeros((NPAD, L), np.float32)   # [node, leaf] +1 where dir=1
    mpath = np.zeros((NPAD, L), np.float32)  # [node, leaf] -1 on path
    for l in range(L):
        for k in range(TREE_DEPTH):
            n = idx[l, k]
            mpath[n, l] -= 1.0
            if dr[l, k]:
                mdir[n, l] += 1.0
    return mdir, mpath


def _pack_consts(split_bias):
    """Build the [128, 512] packed constants array (f32 bits)."""
    consts = np.zeros((128, _CONST_COLS), np.float32)
    # bias columns: bias_pad flattened [(tile, partition)] -> [128, 32]
    bpad = np.zeros((T, NPAD), np.float32)
    bpad[:, :N] = split_bias
    consts[:, _COL_BIAS:_COL_BIAS + NTILES] = bpad.reshape(NTILES, 128).T
    return consts


_NC_CACHE = {}


def _build_bass():
    import concourse.bacc as bacc
    import concourse.mybir as mybir
    import concourse.tile as tile
    from concourse.hw_specs import get_activation_tables

    dt = mybir.dt
    f32 = dt.float32
    f32r = dt.float32r
    fp16 = dt.float16
    AF = mybir.ActivationFunctionType

    nc = bacc.Bacc("TRN2", target_bir_lowering=False, debug=False,
                   num_devices=NCORES)

    # Pin the ACT function table to one containing BOTH Exp and Ln, else the
    # table-load pass ping-pongs between single-function tables.
    table_id = next(i for i, (_, funcs) in
                    enumerate(get_activation_tables("gen3").items())
                    if AF.Exp in funcs and AF.Ln in funcs)
    nc.scalar.add_instruction(mybir.InstLoadActFuncSet(
        name=f"I-{nc.next_id()}", ins=[], outs=[], act_func_set_id=table_id))

    xt = nc.dram_tensor("xt", [D, BS], fp16, kind="ExternalInput").ap()
    wt = nc.dram_tensor("wt", [D, TNP], fp16, kind="ExternalInput").ap()
    consts = nc.dram_tensor("consts", [128, _CONST_COLS], f32r,
                            kind="ExternalInput").ap()
    amat = nc.dram_tensor("amat", [128, 256], fp16, kind="ExternalInput").ap()
    vtf = nc.dram_tensor("vtf", [TNP, C], fp16, kind="ExternalInput").ap()
    out = nc.dram_tensor("out", [C, BS], f32, kind="ExternalOutput").ap()

    NQUAD = 8                     # 8 quads x 4 tiles
    QW = 4 * BS                   # 2048 free columns per quad

    with tile.TileContext(nc) as tc:
        with (
            tc.tile_pool(name="big", bufs=1) as bigp,
            tc.tile_pool(name="const", bufs=1) as constp,
            tc.tile_pool(name="quad", bufs=3) as quadp,
            tc.tile_pool(name="lp", bufs=3) as lpp,
            tc.tile_pool(name="tmp", bufs=2) as tmpp,
            tc.tile_pool(name="pz", bufs=2, space="PSUM") as pzp,
            tc.tile_pool(name="pp", bufs=2, space="PSUM") as ppp,
            tc.tile_pool(name="ps1", bufs=1, space="PSUM") as ps1,
        ):
            # ---- input loads, ordered for earliest PE start -----------
            wt_t = [bigp.tile([128, TNP], fp16, tag=f"wt{j}", name=f"wt{j}")
                    for j in range(4)]
            xt_t = bigp.tile([128, 4 * BS], fp16, tag="xt")
            consts_t = constp.tile([128, _CONST_COLS], f32r, tag="consts")
            vt_all = bigp.tile([128, NTILES * C], fp16, tag="vtall")

            # interleave x chunks with progressively-sized weight chunks
            # across the three DGE queues, ordered by when stage-1 needs
            # them (d-chunk j needs xt chunk j AND wt_t[j] columns).
            qs = [nc.sync, nc.scalar, nc.gpsimd]
            qi = 0

            def dma(out_ap, in_ap):
                nonlocal qi
                qs[qi % 3].dma_start(out=out_ap, in_=in_ap)
                qi += 1

            for j in range(4):
                dma(xt_t[:, j * BS:(j + 1) * BS],
                    xt[j * 128:(j + 1) * 128, :])
                dma(wt_t[j][:, 0:128], wt[j * 128:(j + 1) * 128, 0:128])
            nc.sync.dma_start(out=consts_t[:], in_=consts[:])
            for j in range(4):
                dma(wt_t[j][:, 128:512], wt[j * 128:(j + 1) * 128, 128:512])
            amat_t = constp.tile([128, 256], fp16, tag="amat")
            nc.gpsimd.dma_start(out=amat_t[:], in_=amat[:])
            for cs, ce in [(512, 1536), (1536, 2816), (2816, 4096)]:
                for j in range(4):
                    dma(wt_t[j][:, cs:ce], wt[j * 128:(j + 1) * 128, cs:ce])
            nc.scalar.dma_start(
                out=vt_all[:].rearrange("p (i c) -> p i c", c=C),
                in_=vtf.rearrange("(i p) c -> p i c", p=128),
            )

            adir_ap = amat_t[:, 0:128]
            apath_ap = amat_t[:, 128:256]

            # ---- main pipeline --------------------------------------
            # Units taper from quads to single tiles so the final
            # softplus->pathsum->exp->accumulate chain is short.
            UNITS = [(0, 2), (2, 2), (4, 4), (8, 4), (12, 4), (16, 4),
                     (20, 4), (24, 2), (26, 2), (28, 2), (30, 1), (31, 1)]
            out_ps = ps1.tile([C, BS], f32, tag="outps")
            pending_tail = None
            for u0, un in UNITS:
                ta_q = quadp.tile([128, un * BS], fp16, tag="ta", name="ta")
                for ti in range(un):
                    i = u0 + ti                # tile index
                    pz = pzp.tile([128, BS], f32, tag="pz", name="pz")
                    for j in range(4):
                        nc.tensor.matmul(
                            pz[:],
                            lhsT=wt_t[j][:, i * 128:(i + 1) * 128],
                            rhs=xt_t[:, j * BS:(j + 1) * BS],
                            start=(j == 0), stop=(j == 3),
                        )
                    # Vector: ta tile = pz + bias column
                    nc.vector.tensor_scalar_add(
                        out=ta_q[:, ti * BS:(ti + 1) * BS], in0=pz[:],
                        scalar1=consts_t[:, _COL_BIAS + i:_COL_BIAS + i + 1]
                        .bitcast(f32))

                # unit complete: softplus = Ln(Exp(ta) + 1), one pass each
                te_q = tmpp.tile([128, un * BS], f32, tag="te", name="te")
                nc.scalar.activation(te_q[:], ta_q[:], AF.Exp)
                tb_q = quadp.tile([128, un * BS], fp16, tag="tb", name="tb")
                nc.scalar.activation(tb_q[:], te_q[:], AF.Ln, bias=1.0,
                                     scale=1.0)

                def unit_tail(u0=u0, un=un, ta_q=ta_q, tb_q=tb_q):
                    for pr in range(max(1, un // 2)):
                        i0 = u0 + 2 * pr
                        npr = min(2, un)       # tiles in this pp group
                        pp = ppp.tile([128, npr * BS], f32, tag="pp",
                                      name="pp")
                        for t2 in range(npr):
                            sl = slice(t2 * BS, (t2 + 1) * BS)
                            osl = slice((2 * pr + t2) * BS,
                                        (2 * pr + t2 + 1) * BS)
                            nc.tensor.matmul(pp[:, sl], lhsT=adir_ap,
                                             rhs=ta_q[:, osl],
                                             start=True, stop=False)
                            nc.tensor.matmul(pp[:, sl], lhsT=apath_ap,
                                             rhs=tb_q[:, osl],
                                             start=False, stop=True)
                        lp = lpp.tile([128, npr * BS], fp16, tag="lp",
                                      name="lp")
                        nc.scalar.activation(lp[:], pp[:], AF.Exp)
                        for t2 in range(npr):
                            ii = i0 + t2
                            nc.tensor.matmul(
                                out_ps[:],
                                lhsT=vt_all[:, ii * C:(ii + 1) * C],
                                rhs=lp[:, t2 * BS:(t2 + 1) * BS],
                                start=(ii == 0),
                                stop=(ii == NTILES - 1))

                # defer this unit's path-sum/output matmuls until after the
                # NEXT unit's stage-1 block, so the PE stream never waits on
                # this unit's Ln.
                if pending_tail is not None:
                    pending_tail()
                pending_tail = unit_tail

            if pending_tail is not None:
                pending_tail()

            out_sb = tmpp.tile([C, BS], f32, tag="osb")
            nc.vector.tensor_copy(out=out_sb[:], in_=out_ps[:])
            nc.sync.dma_start(out=out[:], in_=out_sb[:])

    nc.finalize()
    return nc


def _get_nc():
    if "nc" not in _NC_CACHE:
        _NC_CACHE["nc"] = _build_bass()
    return _NC_CACHE["nc"]


def _prep_inputs(x, split_weights, split_bias, leaf_logits, tree_weights):
    x = np.asarray(x, np.float32)
    split_weights = np.asarray(split_weights, np.float32)
    split_bias = np.asarray(split_bias, np.float32)
    leaf_logits = np.asarray(leaf_logits, np.float32)
    tree_weights = np.asarray(tree_weights, np.float32)

    wpad = np.zeros((T, NPAD, D), np.float32)
    wpad[:, :N, :] = split_weights
    wtT = np.ascontiguousarray(
        wpad.reshape(TNP, D).T.astype(np.float16))              # [D, TNP]
    consts = _pack_consts(split_bias)

    # leaf distributions and tree weights are x-independent: fold
    # 2*softmax(tree_weights) * softmax(leaf_logits) on the host.
    tw = tree_weights - tree_weights.max()
    w2 = np.exp(tw)
    w2 = (2.0 / w2.sum()) * w2                                  # [T]
    lle = np.exp(leaf_logits - leaf_logits.max(axis=-1, keepdims=True))
    dist = lle / lle.sum(axis=-1, keepdims=True)                # [T, L, C]
    vtf = np.ascontiguousarray(
        (dist * w2[:, None, None]).reshape(TNP, C).astype(np.float16))

    mdir, mpath = _path_mats()
    amat = np.zeros((128, 256), np.float16)
    amat[:NPAD, 0:L] = mdir
    amat[NPAD:, L:128] = mdir
    amat[:NPAD, 128:128 + L] = mpath
    amat[NPAD:, 128 + L:256] = mpath
    shared = dict(wt=wtT, consts=consts, vtf=vtf, amat=amat)
    in_maps = []
    for i in range(NCORES):
        xti = np.ascontiguousarray(
            x[i * BS:(i + 1) * BS, :].T.astype(np.float16))          # [D, BS]
        in_maps.append(dict(xt=xti, **shared))
    return in_maps


def kernel(x, split_weights, split_bias, leaf_logits, tree_weights):
    from concourse.bass_utils import run_bass_kernel_spmd

    in_maps = _prep_inputs(x, split_weights, split_bias, leaf_logits,
                           tree_weights)
    nc = _get_nc()
    res = run_bass_kernel_spmd(nc, in_maps, core_ids=list(range(NCORES)))
    out = np.concatenate([res.results[i]["out"] for i in range(NCORES)],
                         axis=1).T                              # [B, C]
    return np.ascontiguousarray(out.astype(np.float32))
